# revision 6
# baseline (speedup 1.0000x reference)
"""Trainium2 Bass kernel for nn_AttentionFlow (trilinear attention flow layer).

Full inputs -> shard batch over 8 NeuronCores (2 batches/core) -> gather.

Per batch (C [1024,768], Q [128,768]):
  S[i,j] = w1.C_i + w2.Q_j + (C_i*w3).Q_j
  C2Q = softmax_j(masked S); A = C2Q @ Q
  Q2C = softmax_i(c-masked rowmax of S); Bctx = Q2C @ C
  out = [C | A | C*A | C*Bctx]

v2 design (vs fp32 baseline):
  - All large matmuls run as float32r (PE reads fp32, truncates to fp22,
    1 cycle/row when the moving dim >= 256 vs 4 cycles/row for true fp32).
  - Scores are computed transposed: S^T [m=128 part, n=1024 free] so every
    matmul streams J=512 columns.  lhsT = host-prepared (Q*w3)^T chunks.
  - q_logit ⊗ 1 + 1 ⊗ c_logit is added by ONE K=2 matmul (host-prepared
    rows); c_logit being in S is softmax_j-invariant and makes the row max
    directly equal to the Q2C logit.
  - S^T is copied to SBUF and PE-transposed back per 128-tile for the
    row-space softmax.  exp is shifted by the raw row max (valid upper
    bound); masked-j E values are garbage but are annihilated because the
    A matmul uses qzb = [Q rows zeroed at masked j | (1-qm) column], which
    also yields the softmax normalizer Z as an extra output column.
  - c-masked rows (E==1 everywhere) are fixed to reference semantics
    (uniform over ALL query rows) by one rank-1 matmul adding
    cm ⊗ [sum_masked Q | n_masked] into the A psum.
  - C / A|C*A / C*B are stored by three DMAs per tile; the C part is
    stored straight from the input tile with no staging copy.
"""

from contextlib import ExitStack

import numpy as np

import concourse.bass as bass
import concourse.tile as tile
from concourse import bacc, mybir
from concourse.bass_utils import run_bass_kernel_spmd
from concourse.masks import make_identity

F32 = mybir.dt.float32
F32R = mybir.dt.float32r
AX = mybir.AluOpType
ACTF = mybir.ActivationFunctionType

NEG = np.float32(-1e9)
NCORES = 8
NB = 2           # batches per core
N = 1024         # context length
M = 128          # query length
D = 768          # feature dim
NT = N // 128    # n-tiles per batch
KC = D // 128    # contraction chunks

_CACHE: dict = {}


def _r(ap):
    return ap.bitcast(F32R)


def _build_program(iters: int = 1) -> bass.Bass:
    nc = bacc.Bacc("TRN2", target_bir_lowering=False, debug=False)
    C_d = nc.declare_dram_parameter("C", [NB, N, D], F32R, isOutput=False)
    qzb_d = nc.declare_dram_parameter("qzb", [NB, M, D + 1], F32, isOutput=False)
    qw3_d = nc.declare_dram_parameter("qw3", [NB, KC, 128, M], F32, isOutput=False)
    l2_d = nc.declare_dram_parameter("l2", [NB, 2, M], F32, isOutput=False)
    r2_d = nc.declare_dram_parameter("r2", [NB, 2, N], F32, isOutput=False)
    qcorr_d = nc.declare_dram_parameter("qcorr", [NB, 1, D + 1], F32, isOutput=False)
    cmT_d = nc.declare_dram_parameter("cmT", [NB, 128, NT], F32, isOutput=False)
    cmR_d = nc.declare_dram_parameter("cmR", [NB, 1, N], F32, isOutput=False)
    out_d = nc.declare_dram_parameter("out", [NB, N, 4 * D], F32, isOutput=True)

    with ExitStack() as ctx:
        tc = ctx.enter_context(tile.TileContext(nc))
        consts = ctx.enter_context(tc.tile_pool(name="consts", bufs=1))
        cpool = ctx.enter_context(tc.tile_pool(name="cpool", bufs=2))
        ctpool = ctx.enter_context(tc.tile_pool(name="ctpool", bufs=KC + 2))
        qpool = ctx.enter_context(tc.tile_pool(name="qpool", bufs=2))
        stpool = ctx.enter_context(tc.tile_pool(name="stpool", bufs=4))
        epool = ctx.enter_context(tc.tile_pool(name="epool", bufs=3))
        spool = ctx.enter_context(tc.tile_pool(name="spool", bufs=4))
        stA = ctx.enter_context(tc.tile_pool(name="stA", bufs=3))
        stB = ctx.enter_context(tc.tile_pool(name="stB", bufs=3))
        ps_st = ctx.enter_context(tc.tile_pool(name="ps_st", bufs=2, space="PSUM"))
        ps_t = ctx.enter_context(tc.tile_pool(name="ps_t", bufs=2, space="PSUM"))
        ps_a = ctx.enter_context(tc.tile_pool(name="ps_a", bufs=2, space="PSUM"))

        ident = consts.tile([128, 128], F32)
        make_identity(nc, ident)
        ones_row = consts.tile([1, 128], F32)
        nc.vector.memset(ones_row, 1.0)
        ones_col = consts.tile([128, 1], F32)
        nc.vector.memset(ones_col, 1.0)
        ident_r = consts.tile([128, 128], F32R)
        nc.vector.tensor_copy(out=ident_r, in_=ident)
        ones_row_r = consts.tile([1, 128], F32R)
        nc.vector.tensor_copy(out=ones_row_r, in_=ones_row)

        loop_ctx = tc.For_i(0, iters, 1) if iters > 1 else None
        if loop_ctx is not None:
            ctx.enter_context(loop_ctx)
        for b in range(NB):
            # ---------------- loads (ACT HWDGE ring) ----------------
            c_big = cpool.tile([128, NT, D], F32R, tag="c")
            for t in range(NT):
                nc.scalar.dma_start(
                    out=c_big[:, t, :], in_=C_d[b, t * 128:(t + 1) * 128, :])
            qzb = qpool.tile([128, D + 1], F32, tag="qzb")
            nc.scalar.dma_start(out=qzb, in_=qzb_d[b])
            qw3 = qpool.tile([128, KC, M], F32, tag="qw3")
            nc.scalar.dma_start(out=qw3, in_=qw3_d[b].rearrange("c p m -> p c m"))
            l2 = spool.tile([2, M], F32, tag="l2")
            nc.scalar.dma_start(out=l2, in_=l2_d[b])
            r2 = spool.tile([2, N], F32, tag="r2")
            nc.scalar.dma_start(out=r2, in_=r2_d[b])
            qcorr = spool.tile([1, D + 1], F32, tag="qcorr")
            nc.scalar.dma_start(out=qcorr, in_=qcorr_d[b])
            cmT = spool.tile([128, NT], F32, tag="cmT")
            nc.scalar.dma_start(out=cmT, in_=cmT_d[b])
            cmR = spool.tile([1, N], F32, tag="cmR")
            nc.scalar.dma_start(out=cmR, in_=cmR_d[b])

            # mask derivations: s0=1-cm, negs0=cm-1, cmN=-1e9*cm
            s0c = spool.tile([128, NT], F32, tag="s0c")
            nc.vector.tensor_scalar(out=s0c, in0=cmT, scalar1=-1.0, scalar2=1.0,
                                    op0=AX.mult, op1=AX.add)
            negs0c = spool.tile([128, NT], F32, tag="negs0c")
            nc.vector.tensor_scalar_add(out=negs0c, in0=cmT, scalar1=-1.0)
            cmNc = spool.tile([128, NT], F32, tag="cmNc")
            nc.vector.tensor_scalar_mul(out=cmNc, in0=cmT, scalar1=float(NEG))

            # C part of the output goes out immediately (SP HWDGE ring)
            for t in range(NT):
                nc.sync.dma_start(
                    out=out_d[b, t * 128:(t + 1) * 128, 0:D],
                    in_=c_big[:, t, :].bitcast(F32))

            # ---------------- C^T via PE transposes ----------------
            ct = [ctpool.tile([128, N], F32, tag="ct", name=f"ct{b}_{c}")
                  for c in range(KC)]
            for t in range(NT):
                for c in range(KC):
                    ct_ps = ps_t.tile([128, 128], F32, tag="pst")
                    nc.tensor.transpose(
                        _r(ct_ps), _r(c_big[:, t, c * 128:(c + 1) * 128]),
                        _r(ident))
                    dst = ct[c][:, t * 128:(t + 1) * 128]
                    if (t + c) % 2 == 0:
                        nc.vector.tensor_copy(out=dst, in_=ct_ps)
                    else:
                        nc.scalar.copy(out=dst, in_=ct_ps)

            # ---------------- scores: S^T [m, n] in two 512-halves ----------
            st_sb = [stpool.tile([128, 512], F32R, tag="stsb", name=f"st{b}_{h}")
                     for h in range(2)]
            for h in range(2):
                cols = slice(h * 512, (h + 1) * 512)
                s_ps = ps_st.tile([128, 512], F32, tag="stps")
                for c in range(KC):
                    nc.tensor.matmul(s_ps, lhsT=_r(qw3[:, c, :]),
                                     rhs=_r(ct[c][:, cols]),
                                     start=(c == 0), stop=False)
                nc.tensor.matmul(s_ps, lhsT=_r(l2), rhs=_r(r2[:, cols]),
                                 start=False, stop=True)
                if h == 0:
                    nc.scalar.copy(out=st_sb[h], in_=s_ps)
                else:
                    nc.vector.tensor_copy(out=st_sb[h], in_=s_ps)

            # ---------------- per n-tile: softmax row space + A ----------
            G = spool.tile([128, NT], F32, tag="G")
            for t in range(NT):
                blk_ps = ps_t.tile([128, 128], F32, tag="pst")
                src = st_sb[t // 4][:, (t % 4) * 128:(t % 4 + 1) * 128]
                nc.tensor.transpose(_r(blk_ps), src, ident_r)
                nrawmax = spool.tile([128, 1], F32, tag="nrawmax")
                nc.vector.reduce_max(out=nrawmax, in_=blk_ps,
                                     axis=mybir.AxisListType.X, negate=True)
                # Q2C logit: G = rawmax*s0 - 1e9*cm  (rawmax includes c_logit)
                nc.vector.tensor_scalar(out=G[:, t:t + 1], in0=nrawmax,
                                        scalar1=negs0c[:, t:t + 1],
                                        scalar2=cmNc[:, t:t + 1],
                                        op0=AX.mult, op1=AX.add)
                biasT = spool.tile([128, 1], F32, tag="biasT")
                nc.vector.tensor_scalar_mul(out=biasT, in0=nrawmax,
                                            scalar1=s0c[:, t:t + 1])
                E = epool.tile([128, M], F32R, tag="E")
                nc.scalar.activation(out=E, in_=blk_ps, func=ACTF.Exp,
                                     bias=biasT, scale=s0c[:, t:t + 1])
                et_ps = ps_t.tile([128, M], F32, tag="pst")
                nc.tensor.transpose(_r(et_ps), E, ident_r)
                et = epool.tile([128, M], F32, tag="et")
                nc.vector.tensor_copy(out=et, in_=et_ps)

                a_ps = ps_a.tile([128, D + 2], F32, tag="aps")
                cmr_t = cmR[:, t * 128:(t + 1) * 128]
                nc.tensor.matmul(a_ps[:, 0:512], lhsT=_r(et), rhs=_r(qzb[:, 0:512]),
                                 start=True, stop=False)
                nc.tensor.matmul(a_ps[:, 0:512], lhsT=_r(cmr_t),
                                 rhs=_r(qcorr[:, 0:512]), start=False, stop=True)
                nc.tensor.matmul(a_ps[:, 512:D + 1], lhsT=_r(et),
                                 rhs=_r(qzb[:, 512:D + 1]), start=True, stop=False)
                nc.tensor.matmul(a_ps[:, 512:D + 1], lhsT=_r(cmr_t),
                                 rhs=_r(qcorr[:, 512:D + 1]), start=False, stop=True)
                zr = spool.tile([128, 1], F32, tag="zr")
                nc.vector.reciprocal(out=zr, in_=a_ps[:, D:D + 1])

                stage = stA.tile([128, 2 * D], F32, tag="stA")
                if t % 2 == 0:
                    nc.scalar.activation(out=stage[:, 0:D], in_=a_ps[:, 0:D],
                                         func=ACTF.Copy, scale=zr)
                else:
                    nc.vector.tensor_scalar_mul(out=stage[:, 0:D],
                                                in0=a_ps[:, 0:D], scalar1=zr)
                ca_eng = nc.gpsimd if t % 2 == 0 else nc.vector
                ca_eng.tensor_mul(out=stage[:, D:2 * D], in0=stage[:, 0:D],
                                  in1=c_big[:, t, :].bitcast(F32))
                nc.sync.dma_start(
                    out=out_d[b, t * 128:(t + 1) * 128, D:3 * D], in_=stage)

            # ---------------- Q2C global softmax + Bctx ----------------
            gt_ps = ps_t.tile([NT, 128], F32, tag="pst")
            nc.tensor.transpose(gt_ps, G, ident)
            m8n = spool.tile([NT, 1], F32, tag="m8n")
            nc.vector.reduce_max(out=m8n, in_=gt_ps, axis=mybir.AxisListType.X,
                                 negate=True)
            m8t_ps = ps_t.tile([1, NT], F32, tag="pst")
            nc.tensor.transpose(m8t_ps, m8n, ident[0:NT, 0:NT])
            negMg = spool.tile([1, 1], F32, tag="negMg")
            nc.vector.tensor_reduce(out=negMg, in_=m8t_ps,
                                    axis=mybir.AxisListType.X, op=AX.min)
            nm8_ps = ps_t.tile([NT, 1], F32, tag="pst")
            nc.tensor.matmul(nm8_ps, lhsT=ones_row[:, 0:NT], rhs=negMg,
                             start=True, stop=True)
            nm8 = spool.tile([NT, 1], F32, tag="nm8")
            nc.vector.tensor_copy(out=nm8, in_=nm8_ps)
            er8 = spool.tile([NT, 128], F32, tag="er8")
            zq8 = spool.tile([NT, 1], F32, tag="zq8")
            nc.scalar.activation(out=er8, in_=gt_ps, func=ACTF.Exp, bias=nm8,
                                 accum_out=zq8)
            zq_ps = ps_t.tile([1, 1], F32, tag="pst")
            nc.tensor.matmul(zq_ps, lhsT=zq8, rhs=ones_col[0:NT, :],
                             start=True, stop=True)
            zqr = spool.tile([1, 1], F32, tag="zqr")
            nc.vector.reciprocal(out=zqr, in_=zq_ps)
            ec_ps = ps_t.tile([128, NT], F32, tag="pst")
            nc.tensor.transpose(ec_ps, er8, ident[0:NT, 0:NT])
            ecol = spool.tile([128, NT], F32R, tag="ecol")
            nc.vector.tensor_copy(out=ecol, in_=ec_ps)
            bctx_ps = ps_a.tile([1, D], F32, tag="aps")
            for t in range(NT):
                nc.tensor.matmul(bctx_ps[:, 0:512], lhsT=_r(ecol[:, t:t + 1]),
                                 rhs=_r(c_big[:, t, 0:512]), start=(t == 0),
                                 stop=(t == NT - 1))
                nc.tensor.matmul(bctx_ps[:, 512:D], lhsT=_r(ecol[:, t:t + 1]),
                                 rhs=_r(c_big[:, t, 512:D]), start=(t == 0),
                                 stop=(t == NT - 1))
            bctx = spool.tile([1, D], F32R, tag="bctx")
            nc.scalar.activation(out=bctx, in_=bctx_ps, func=ACTF.Copy, scale=zqr)
            bb_ps = ps_a.tile([128, D], F32, tag="aps")
            nc.tensor.matmul(bb_ps[:, 0:512], lhsT=_r(ones_row),
                             rhs=_r(bctx[:, 0:512]), start=True, stop=True)
            nc.tensor.matmul(bb_ps[:, 512:D], lhsT=_r(ones_row),
                             rhs=_r(bctx[:, 512:D]), start=True, stop=True)
            Bb = qpool.tile([128, D], F32, tag="Bb")
            nc.scalar.copy(out=Bb, in_=bb_ps)
            for t in range(NT):
                sb = stB.tile([128, D], F32, tag="stB")
                cb_eng = nc.gpsimd if t % 2 == 1 else nc.vector
                cb_eng.tensor_mul(out=sb, in0=c_big[:, t, :].bitcast(F32), in1=Bb)
                nc.sync.dma_start(
                    out=out_d[b, t * 128:(t + 1) * 128, 3 * D:4 * D], in_=sb)
    nc.compile()
    return nc


def _get_program() -> bass.Bass:
    if "nc" not in _CACHE:
        _CACHE["nc"] = _build_program()
    return _CACHE["nc"]


def _make_in_maps(inputs) -> list:
    C = np.ascontiguousarray(np.asarray(inputs["C"], dtype=np.float32))
    Q = np.ascontiguousarray(np.asarray(inputs["Q"], dtype=np.float32))
    c_mask = np.asarray(inputs["c_mask"])
    q_mask = np.asarray(inputs["q_mask"])
    w1 = np.asarray(inputs["w1"], dtype=np.float32).reshape(-1)
    w2 = np.asarray(inputs["w2"], dtype=np.float32).reshape(-1)
    w3 = np.asarray(inputs["w3"], dtype=np.float32).reshape(-1)
    B = C.shape[0]

    qm = q_mask[:, 0, :].astype(np.float32)                     # [B,M] 1=masked
    qbin = 1.0 - qm
    qzb = np.concatenate([Q * qbin[:, :, None], qbin[:, :, None],
                      np.zeros((B, M, 1), np.float32)], axis=2)
    qw3 = np.ascontiguousarray(
        (Q * w3.reshape(1, 1, D)).transpose(0, 2, 1).reshape(B, KC, 128, M))
    qlogit = Q @ w2                                             # [B,M]
    l2 = np.ascontiguousarray(
        np.stack([qlogit, np.ones_like(qlogit)], axis=1))       # [B,2,M]
    clogit = C @ w1                                             # [B,N]
    r2 = np.ascontiguousarray(
        np.stack([np.ones_like(clogit), clogit], axis=1))       # [B,2,N]
    qcorr = np.ascontiguousarray(np.concatenate(
        [np.einsum('bm,bmd->bd', qm, Q), qm.sum(1, keepdims=True),
         np.zeros((B, 1), np.float32)], axis=1).reshape(B, 1, D + 2))
    cmf = c_mask[:, 0, :].astype(np.float32)                    # [B,N]
    cmT = np.ascontiguousarray(cmf.reshape(B, NT, 128).transpose(0, 2, 1))
    cmR = np.ascontiguousarray(cmf.reshape(B, 1, N))

    in_maps = []
    for core in range(NCORES):
        sl = slice(core * NB, (core + 1) * NB)
        in_maps.append({
            "C": C[sl],
            "qzb": np.ascontiguousarray(qzb[sl]),
            "qw3": np.ascontiguousarray(qw3[sl]),
            "l2": np.ascontiguousarray(l2[sl]),
            "r2": np.ascontiguousarray(r2[sl]),
            "qcorr": np.ascontiguousarray(qcorr[sl]),
            "cmT": np.ascontiguousarray(cmT[sl]),
            "cmR": np.ascontiguousarray(cmR[sl]),
        })
    return in_maps


def kernel(**inputs) -> np.ndarray:
    nc = _get_program()
    in_maps = _make_in_maps(inputs)
    res = run_bass_kernel_spmd(nc, in_maps, list(range(NCORES)))
    return np.concatenate([r["out"] for r in res.results], axis=0)
